# revision 40
# baseline (speedup 1.0000x reference)
"""Hybrid-sharded per-position linear: 4 batch quarters x 2 sequence halves.

Per core: 1024 batches x 21 owned positions (+2 halo x slices, zero-fed at
global edges; half 1's 21st position is a discarded pad). x ships as fp8
e3m4 (total l2 rel err 0.0135 incl f16 out, vs the 2e-2 budget); W stays
bf16 -- the PE takes mixed fp8-stationary x bf16-moving operands. Bytes per
core: x 3.01MB + W 2.06MB + out f16 5.51MB = 10.6MB (~29us of DMA at
360GB/s), under the PE floor, so the kernel is tensor-bound.

PE floor: 64512 moving columns = 26.9us at the full 2.4GHz clock. The
schedule hits it exactly:
- ~26 dummy matmuls on a zeroed SBUF tile bridge t~0.9us to the first real
  matmul so the PE p-state ramp (0.65/1.2GHz for the first 3us of a busy
  streak) completes before real work; the real stream then runs entirely
  at 2.4GHz with zero gaps.
- Loads split across sync/HWDGE (W group 0 first half, x slices 0-1, 3-5,
  W groups 1-5) and Pool/SWDGE (x slice 2, rest of W0, x groups 1-4);
  group 0 accumulates in two PSUM passes (ents 0-2, then 3-5 per tile) to
  track arrival order. First matmul at ~4.2us, bounded by DGE pipeline
  latency (~1.9us) + first chunks + the 900ns DMA-completion sem.
- PSUM tiles are single batch-subtile [128, 4*128] = 1 bank, so the pool
  rotates 8-deep and the relu chain (mm -> sem -> relu -> sem, ~2.7us)
  never back-pressures the PE through the 4-deep wait queue.
- ReLU+f16 downcast alternates DVE / ACT per tile (one engine alone is
  ~25us of relu); the ACT Relu table load is pulled to t~0 by a dummy
  activation. The final position (21st) is computed as 4 two-subtile PSUM
  tiles spread across the last groups so the stream ends on a tiny
  matmul->relu->store chain; stores for the last group are split so each
  piece leaves right behind its relu.

Tail after the last matmul (~4.7us) is structural and sem-bound: mm->relu
sem, the packed end relu queue, store issue (HWDGE 625 + DGE 650),
transfer, 900ns DMA sem, engine drain barrier. One store per main group
(NSPLIT_MAIN=1) keeps SP sequencer slots clear ahead of the tail stores.
"""

import os
import sys

import numpy as np
import ml_dtypes

for _p in ("/opt/trn_rl_repo", "/root/.axon_site/_ro/trn_rl_repo"):
    if os.path.isdir(_p) and _p not in sys.path:
        sys.path.append(_p)

from contextlib import ExitStack

import concourse.mybir as mybir
import concourse.tile as tile
from concourse import bacc
from concourse.bass_utils import run_bass_kernel_spmd

S = 41
F = 128
WIN = 3
N_CORES = 8
B_FULL = 4096

SH = 2                    # sequence halves
PO = 21                   # owned positions per half (half 1: 20 real + 1 pad)
XS = PO + 2               # x slices incl halo
BQ = 4                    # batch quarters
NB = B_FULL // BQ         # 1024 batches per core
NBT = NB // 128           # 8 batch sub-tiles

_XGROUPS = [(0, 6), (6, 4), (10, 4), (14, 4), (18, 5)]  # (slice0, nslices)
_SLICE_MAP = {}
for _gi, (_s0, _n) in enumerate(_XGROUPS):
    for _k in range(_n):
        _SLICE_MAP[_s0 + _k] = (_gi, _k)

N_WARM = 26
RELU_PARITY = 0
FINAL_RELU = 0
NSPLIT_MAIN = 1
LAST_Q_MERGE = 0

_nc_cache = {}


_GROUPDEF = [(0, 4), (4, 4), (8, 4), (12, 4), (16, 4), (20, 1)]


def _layout2():
    """Matmul groups over local positions 0..PO-1. Entry (i, jmin, ncons):
    stationary x-slice i (local; slice i = global h*PO-1+i), consumers
    positions j in [jmin, jmin+ncons) with tap w = i - j."""
    out = []
    for j0, n in _GROUPDEF:
        ents = []
        for i in range(j0, j0 + n + 2):
            jmin = max(j0, i - 2)
            jmax = min(j0 + n - 1, i)
            if jmax >= jmin:
                ents.append((i, jmin, jmax - jmin + 1))
        out.append((j0, n, ents))
    return out


_LAYOUT = _layout2()
_WCOLS = [sum(nc_ * F for _, _, nc_ in ents) for _, _, ents in _LAYOUT]
_WTOT = sum(_WCOLS)
_NG = len(_LAYOUT)  # 6: five 4-position groups + final 1-position group


def _build(has_bias: bool):
    bf16 = mybir.dt.bfloat16
    f32 = mybir.dt.float32
    f16 = mybir.dt.float16
    e3m4 = mybir.dt.float8e3
    nc = bacc.Bacc("TRN2", target_bir_lowering=False, debug=False)
    xT = nc.dram_tensor("xT", [XS, F, NB], e3m4, kind="ExternalInput").ap()
    Wg = nc.dram_tensor("Wg", [F, _WTOT], bf16, kind="ExternalInput").ap()
    bias = (
        nc.dram_tensor("bias", [1, PO * F], bf16, kind="ExternalInput").ap()
        if has_bias
        else None
    )
    out = nc.dram_tensor("out", [NB, PO, F], f16, kind="ExternalOutput").ap()

    def relu(dst, src, eng):
        if eng == 0:
            nc.vector.tensor_scalar_max(dst, src, 0.0)
        else:
            nc.scalar.activation(dst, src, mybir.ActivationFunctionType.Relu)

    with tile.TileContext(nc) as tc:
        with ExitStack() as ctx:
            xpool = ctx.enter_context(tc.tile_pool(name="xT", bufs=1))
            wpool = ctx.enter_context(tc.tile_pool(name="W", bufs=1))
            ppool = ctx.enter_context(tc.tile_pool(name="ps", bufs=8, space="PSUM"))
            opool = ctx.enter_context(tc.tile_pool(name="stage", bufs=_NG - 1))
            tpool = ctx.enter_context(tc.tile_pool(name="tail", bufs=4))

            wpool1 = ctx.enter_context(tc.tile_pool(name="warm", bufs=1))
            warm_in = wpool1.tile([1, 2], f32)
            warm_out = wpool1.tile([1, 2], f32)
            wsrc = wpool1.tile([128, 128], bf16, name="wsrc")
            nc.vector.memset(wsrc[:], 0.0)
            nc.vector.memset(warm_in[:], 0.0)
            # pulls the Relu act table load to t~0, off the critical path
            nc.scalar.activation(
                warm_out[:], warm_in[:], mybir.ActivationFunctionType.Relu
            )
            # dummy matmuls keep the PE busy from ~0.9us so the 2.4GHz
            # p-state is reached before the first real matmul issues; the
            # real stream then runs entirely at full clock
            ps_warm = ppool.tile([128, 128], f32, name="ps")
            for _ in range(N_WARM):
                nc.tensor.matmul(
                    ps_warm[:], lhsT=wsrc[:], rhs=wsrc[:], start=True, stop=True
                )

            xt = [xpool.tile([F, n * NB], e3m4, name=f"x{i}") for i, (_, n) in enumerate(_XGROUPS)]
            wt = [wpool.tile([F, _WCOLS[g]], bf16, name=f"w{g}") for g in range(_NG)]

            # first-matmul gate = W0a + slice 0. The pool queue's first DMA
            # beats the sync queue's second onto the bus (SWDGE generation is
            # quicker than a second HWDGE pass), so slice 2 rides there.
            x0 = xt[0][:].rearrange("k (s b) -> k s b", b=NB)
            half0 = sum(nc_ for _, _, nc_ in _LAYOUT[0][2][:3]) * F  # passA cols
            wc_of_g = [sum(_WCOLS[:g]) for g in range(_NG)]

            def xg_load(eng, gi):
                s0_, ns = _XGROUPS[gi]
                eng.dma_start(
                    xt[gi][:].rearrange("k (s b) -> k s b", b=NB)[:, :ns, :],
                    xT[s0_ : s0_ + ns].rearrange("s k b -> k s b"),
                )

            def w_load(eng, g):
                eng.dma_start(
                    wt[g][:], Wg[:, wc_of_g[g] : wc_of_g[g] + _WCOLS[g]]
                )

            # sync/HWDGE: W0 first half, x slices 0-1, x slices 3-5, W1-5
            nc.sync.dma_start(wt[0][:, :half0], Wg[:, :half0])
            for sl in range(2):
                nc.sync.dma_start(
                    x0[:, sl : sl + 1, :],
                    xT[sl : sl + 1].rearrange("s k b -> k s b"),
                )
            nc.sync.dma_start(x0[:, 3:6, :], xT[3:6].rearrange("s k b -> k s b"))
            for g in range(1, _NG):
                w_load(nc.sync, g)
            # pool/SWDGE: x slice 2, W0 second half, x groups 1-4
            nc.gpsimd.dma_start(
                x0[:, 2:3, :], xT[2:3].rearrange("s k b -> k s b")
            )
            nc.gpsimd.dma_start(wt[0][:, half0 : _WCOLS[0]], Wg[:, half0 : _WCOLS[0]])
            for gi in range(1, len(_XGROUPS)):
                xg_load(nc.gpsimd, gi)

            if has_bias:
                bpool = ctx.enter_context(tc.tile_pool(name="bias", bufs=1))
                bias_sb = bpool.tile([1, PO * F], bf16)
                nc.sync.dma_start(bias_sb[:], bias[:])
                ones = bpool.tile([1, F], bf16)
                nc.vector.memset(ones[:], 1.0)

            out_r = out.rearrange("(t p) s f -> p t s f", p=128)

            def lhsT(si, bt):
                gi, sub = _SLICE_MAP[si]
                return xt[gi][:, sub * NB + bt * 128 : sub * NB + (bt + 1) * 128]

            s5, _, ents5 = _LAYOUT[_NG - 1]
            w5_of, wc5 = [], 0
            for si, jmin, ncons in ents5:
                w5_of.append(wc5)
                wc5 += ncons * F
            n_mm5 = len(ents5) + (1 if has_bias else 0)

            def emit_tail(bt0, eng):
                """Two-subtile chain for the final position: 6 short matmuls,
                one small relu, one small store. Spread across groups 2-4 so
                only the last chain trails the final matmul."""
                ps = ppool.tile([128, 2 * F], f32, name="ps")
                for th in range(2):
                    for j, (si, _, _) in enumerate(ents5):
                        nc.tensor.matmul(
                            ps[:, th * F : (th + 1) * F],
                            lhsT=lhsT(si, bt0 + th),
                            rhs=wt[_NG - 1][:, w5_of[j] : w5_of[j] + F],
                            start=(j == 0),
                            stop=(j == n_mm5 - 1),
                        )
                    if has_bias:
                        nc.tensor.matmul(
                            ps[:, th * F : (th + 1) * F],
                            lhsT=ones[:],
                            rhs=bias_sb[:, s5 * F : (s5 + 1) * F],
                            start=False,
                            stop=True,
                        )
                ts = tpool.tile([128, 2 * F], f16, name="tstage")
                relu(ts[:], ps[:], eng)

                def _store(ts=ts, bt0=bt0):
                    nc.sync.dma_start(
                        out_r[:, bt0 : bt0 + 2, s5 : s5 + 1, :],
                        ts[:].rearrange("p (t s f) -> p t s f", t=2, s=1),
                    )

                return _store

            # --- groups 0..4: four positions per PSUM tile ---
            for g in range(_NG - 1):
                s0, npos, ents = _LAYOUT[g]
                stage = opool.tile([128, NBT * npos * F], f16, tag="stage")
                stage_c = stage[:].rearrange("p (t c) -> p t c", t=NBT)
                n_mm = len(ents) + (1 if has_bias else 0)

                # group 0 runs ents 0-2 over all tiles first, then ents 3-5,
                # tracking the two W chunk arrivals
                tail_stores = []
                if g == 0:
                    passes = [(0, 3), (3, len(ents))]
                else:
                    passes = [(0, len(ents))]
                tiles = []
                wcol_of = []
                wc = 0
                for si, jmin, ncons in ents:
                    wcol_of.append(wc)
                    wc += ncons * F
                for pi, (e0, e1) in enumerate(passes):
                    for bt in range(NBT):
                        if pi == 0:
                            ps = ppool.tile([128, npos * F], f32)
                            tiles.append(ps)
                        else:
                            ps = tiles[bt]
                        for j in range(e0, e1):
                            si, jmin, ncons = ents[j]
                            c0 = (jmin - s0) * F
                            nc.tensor.matmul(
                                ps[:, c0 : c0 + ncons * F],
                                lhsT=lhsT(si, bt),
                                rhs=wt[g][:, wcol_of[j] : wcol_of[j] + ncons * F],
                                start=(j == 0),
                                stop=(j == n_mm - 1),
                            )
                        if has_bias and e1 == len(ents):
                            nc.tensor.matmul(
                                ps[:, : npos * F],
                                lhsT=ones[:],
                                rhs=bias_sb[:, s0 * F : (s0 + npos) * F],
                                start=False,
                                stop=True,
                            )
                        if e1 == len(ents):
                            # stream-ending tile rides the faster DVE sem path
                            final_tile = g == _NG - 2 and bt == NBT - 1
                            pv = ps[:].rearrange("p (h c) -> p h c", h=1)
                            if final_tile and FINAL_RELU == 1:
                                hw_ = npos * F // 2
                                sc = stage_c[:, bt : bt + 1, :]
                                relu(sc[:, :, :hw_], pv[:, :, :hw_], eng=0)
                                relu(
                                    sc[:, :, hw_ : npos * F],
                                    pv[:, :, hw_ : npos * F],
                                    eng=1,
                                )
                            else:
                                feng = (
                                    (0 if FINAL_RELU == 0 else 1)
                                    if final_tile
                                    else (bt + RELU_PARITY) % 2
                                )
                                relu(
                                    stage_c[:, bt : bt + 1, : npos * F],
                                    pv[:, :, : npos * F],
                                    eng=feng,
                                )
                            if g == _NG - 2 and bt == 3:
                                tail_stores.append(emit_tail(4, 0))
                            elif g == _NG - 2 and bt == NBT - 1:
                                tail_stores.append(emit_tail(6, 0))
                if g == _NG - 4:
                    emit_tail(0, 0)()
                elif g == _NG - 3:
                    emit_tail(2, 1)()
                stage_v = stage[:].rearrange("p (t s f) -> p t s f", t=NBT, f=F)
                if g == _NG - 2:
                    # last group: pair stores, the final pair split per half,
                    # and the deferred tail-tile stores slotted in expected
                    # sem-arrival order (the SP wait queue drains in order)
                    if LAST_Q_MERGE:
                        nc.sync.dma_start(
                            out_r[:, 0:4, s0 : s0 + npos, :],
                            stage_v[:, 0:4, :npos, :],
                        )
                    else:
                        for o in range(2):
                            nc.sync.dma_start(
                                out_r[:, o * 2 : o * 2 + 2, s0 : s0 + npos, :],
                                stage_v[:, o * 2 : o * 2 + 2, :npos, :],
                            )
                    tail_stores[0]()
                    nc.sync.dma_start(
                        out_r[:, 4:6, s0 : s0 + npos, :],
                        stage_v[:, 4:6, :npos, :],
                    )
                    for t in (6, 7):
                        nc.sync.dma_start(
                            out_r[:, t : t + 1, s0 : s0 + npos, :],
                            stage_v[:, t : t + 1, :npos, :],
                        )
                    tail_stores[1]()
                else:
                    h = NBT // NSPLIT_MAIN
                    for o in range(NSPLIT_MAIN):
                        nc.sync.dma_start(
                            out_r[:, o * h : (o + 1) * h, s0 : s0 + npos, :],
                            stage_v[:, o * h : (o + 1) * h, :npos, :],
                        )

    nc.compile()
    return nc


def _get_nc(has_bias: bool):
    if has_bias not in _nc_cache:
        _nc_cache[has_bias] = _build(has_bias)
    return _nc_cache[has_bias]


def _prep_in_maps(inputs: np.ndarray, W: np.ndarray, b: np.ndarray):
    has_bias = bool(np.any(b))
    xb = inputs.astype(ml_dtypes.float8_e3m4)
    Wb = W.astype(ml_dtypes.bfloat16)
    wgs, biases = [], []
    for h in range(SH):
        valid = min(PO, S - h * PO)
        Wh = np.zeros((PO, WIN * F, F), ml_dtypes.bfloat16)
        Wh[:valid] = Wb[h * PO : h * PO + valid]
        blocks = []
        for s0, npos, ents in _LAYOUT:
            for si, jmin, ncons in ents:
                for j in range(jmin, jmin + ncons):
                    w = si - j  # tap: slice si = position si-1 locally
                    blocks.append(Wh[j, w * F : (w + 1) * F, :])
        wgs.append(np.ascontiguousarray(np.concatenate(blocks, axis=1)))
        assert wgs[-1].shape == (F, _WTOT)
        if has_bias:
            bh = np.zeros((PO, F), ml_dtypes.bfloat16)
            bh[:valid] = b[h * PO : h * PO + valid].astype(ml_dtypes.bfloat16)
            biases.append(np.ascontiguousarray(bh.reshape(1, PO * F)))

    in_maps = []
    for c in range(N_CORES):
        h, bp = divmod(c, BQ)
        g0 = h * PO - 1  # global position of local x slice 0
        xs = np.zeros((XS, F, NB), ml_dtypes.float8_e3m4)
        glo, ghi = max(0, g0), min(S - 1, g0 + XS - 1)
        xs[glo - g0 : ghi - g0 + 1] = xb[
            bp * NB : (bp + 1) * NB, glo : ghi + 1, :
        ].transpose(1, 2, 0)
        m = {"xT": np.ascontiguousarray(xs), "Wg": wgs[h]}
        if has_bias:
            m["bias"] = biases[h]
        in_maps.append(m)
    return in_maps, has_bias


def kernel(inputs: np.ndarray, W: np.ndarray, b: np.ndarray) -> np.ndarray:
    inputs = np.asarray(inputs)
    W = np.asarray(W)
    b = np.asarray(b)
    assert inputs.shape == (B_FULL, S, F), inputs.shape
    in_maps, has_bias = _prep_in_maps(inputs, W, b)
    nc = _get_nc(has_bias)
    res = run_bass_kernel_spmd(nc, in_maps, list(range(N_CORES)))
    out = np.empty((B_FULL, S, F), np.float32)
    for c in range(N_CORES):
        h, bp = divmod(c, BQ)
        valid = min(PO, S - h * PO)
        out[bp * NB : (bp + 1) * NB, h * PO : h * PO + valid, :] = (
            res.results[c]["out"][:, :valid, :].astype(np.float32)
        )
    return out
